# revision 1
# baseline (speedup 1.0000x reference)
"""GNN message-passing kernel for Trainium2 (8 NeuronCores, SPMD).

Computation (see reference):
  h1 = tanh(segsum(x[src] -> dst) @ W1 + b1)        [uses A(xW) = (Ax)W]
  h2 = tanh(segsum(h1[src] -> dst) @ W2 + b2)
  ht = logmap0(proj(h2))  (rowwise scale)
  pooled = segment mean over seg_ids, then expmap0/proj (host epilogue)

Sharding: nodes split contiguously over cores (dst-shard). Each core owns
SHARD nodes, processes the edges whose dst is in its shard.  The spmm is a
one-hot matmul: for each 128-edge tile, S^T[e,slot] = (dstslot[e]==slot)
(DVE is_equal vs iota), stationary lhsT=S^T, moving rhs = gathered rows.
Gather via gpsimd.dma_gather with int16 indices (tables chunked to 32768
rows).  The only cross-core exchange is one AllGather of h1 (bf16).
"""

import math
from contextlib import ExitStack

import numpy as np
import ml_dtypes

import concourse.bass as bass
import concourse.tile as tile
import concourse.bacc as bacc
from concourse import mybir

BF16 = mybir.dt.bfloat16
F32 = mybir.dt.float32
I16 = mybir.dt.int16
AF = mybir.ActivationFunctionType
ALU = mybir.AluOpType

MAXNORM = 1.0 - 1e-5
MIN_SS = 1e-15

SUB = 1024          # gather indices per dma_gather call (descriptor ring limit)
GRP = 4             # dst blocks (of 128 nodes) per PSUM group


class Cfg:
    def __init__(self, n_nodes, in_dim, hid, n_seg, n_cores):
        self.N = n_nodes
        self.IN = in_dim
        self.HID = hid
        self.NSEG = n_seg
        self.NC = n_cores
        self.SHARD = n_nodes // n_cores
        assert self.SHARD % 128 == 0
        self.NBLK = self.SHARD // 128
        assert self.NBLK % GRP == 0
        self.NGRP = self.NBLK // GRP
        self.CH = min(32768, n_nodes)
        assert n_nodes % self.CH == 0
        self.NCHUNK = n_nodes // self.CH
        self.NSEGCH = (n_seg + 127) // 128


def host_prep(cfg, src, dst):
    """Build SPMD-uniform edge tiling + per-core index/slot arrays.

    Returns (ntiles[NGRP,NCHUNK,GRP], per-core list of dicts with
    idx16 [128, TOT/16] int16 and dstslot [128, NTILES] float arrays).
    """
    NC, SHARD, CH = cfg.NC, cfg.SHARD, cfg.CH
    src = np.asarray(src).astype(np.int64)
    dst = np.asarray(dst).astype(np.int64)

    core = dst // SHARD
    blk = (dst % SHARD) // 128          # block within core [0, NBLK)
    slot = dst % 128
    chunk = src // CH
    idx = src % CH

    # counts[c, g, k, b]
    counts = np.zeros((NC, cfg.NGRP, cfg.NCHUNK, GRP), dtype=np.int64)
    g_all = blk // GRP
    b_all = blk % GRP
    np.add.at(counts, (core, g_all, chunk, b_all), 1)

    mx = counts.max(axis=0)
    ntiles = (mx + 127) // 128
    # ensure every block has >= 1 tile in chunk 0 (so PSUM gets a start write)
    empty = ntiles.sum(axis=1) == 0      # [NGRP, GRP]
    ntiles[:, 0, :][empty] = 1

    NTILES = int(ntiles.sum())
    TOT = NTILES * 128

    per_core = []
    # canonical ordering: g, k, b, then edges of that cell (+pad)
    order = np.lexsort((idx, b_all, chunk, g_all, core))
    # cell boundaries per core
    for c in range(NC):
        idx16 = np.zeros(TOT, dtype=np.int16)
        slots = np.full(TOT, -1.0, dtype=np.float32)
        sel = order[core[order] == c]
        csrc_idx = idx[sel]
        cslot = slot[sel]
        cg = g_all[sel]
        ck = chunk[sel]
        cb = b_all[sel]
        # counts per cell for this core
        ccnt = counts[c]
        pos = 0      # position in canonical padded stream
        ep = 0       # position in sel
        for g in range(cfg.NGRP):
            for k in range(cfg.NCHUNK):
                for b in range(GRP):
                    n = int(ccnt[g, k, b])
                    cap = int(ntiles[g, k, b]) * 128
                    if n > 0:
                        idx16[pos:pos + n] = csrc_idx[ep:ep + n]
                        slots[pos:pos + n] = cslot[ep:ep + n]
                        # sanity
                        assert np.all(cg[ep:ep + n] == g)
                        assert np.all(ck[ep:ep + n] == k)
                        assert np.all(cb[ep:ep + n] == b)
                        ep += n
                    pos += cap
        assert ep == len(sel)
        # wrap idx: i -> [i%16, i//16], replicate x8 partitions
        iw = idx16.reshape(-1, 16).T            # [16, TOT/16]
        iw = np.tile(iw, (8, 1)).copy()         # [128, TOT/16]
        # dstslot tile-major: [128 (edge in tile), NTILES]
        sl = slots.reshape(NTILES, 128).T.copy()
        per_core.append({"idx16": iw.astype(np.int16),
                         "dstslot": sl.astype(np.float32)})
    return ntiles, per_core


def _mm_schedule(cfg, ntiles):
    """Per (g): list over chunks of list of (tile_global_col, block b, start, stop)."""
    sched = []
    tcol = 0
    for g in range(cfg.NGRP):
        # first/last tile of each block across chunks
        tot_b = ntiles[g].sum(axis=0)   # [GRP]
        seen_b = np.zeros(GRP, dtype=np.int64)
        chunks = []
        for k in range(cfg.NCHUNK):
            tiles_k = []
            for b in range(GRP):
                for _ in range(int(ntiles[g, k, b])):
                    start = seen_b[b] == 0
                    stop = seen_b[b] == tot_b[b] - 1
                    tiles_k.append((tcol, b, bool(start), bool(stop)))
                    seen_b[b] += 1
                    tcol += 1
            chunks.append(tiles_k)
        sched.append(chunks)
    return sched


def build(cfg, ntiles, n_reps=1, debug_taps=False):
    """Build the Bass program. Returns nc."""
    N, IN, HID = cfg.N, cfg.IN, cfg.HID
    NTILES = int(ntiles.sum())
    TOT = NTILES * 128
    sched = _mm_schedule(cfg, ntiles)

    nc = bacc.Bacc("TRN2", target_bir_lowering=False)

    x_d = nc.dram_tensor("x_bf16", [N, IN], BF16, kind="ExternalInput")
    idx_d = nc.dram_tensor("idx16", [128, TOT // 16], I16, kind="ExternalInput")
    slot_d = nc.dram_tensor("dstslot", [128, NTILES], F32, kind="ExternalInput")
    segid_d = nc.dram_tensor("segid", [128, cfg.NBLK], F32, kind="ExternalInput")
    iota_d = nc.dram_tensor("iota128", [128, 128], BF16, kind="ExternalInput")
    iotas_d = nc.dram_tensor("iota_seg", [128, cfg.NSEGCH * 128], F32, kind="ExternalInput")
    ident_d = nc.dram_tensor("ident", [128, 128], BF16, kind="ExternalInput")
    w1_d = nc.dram_tensor("W1", [IN, HID], BF16, kind="ExternalInput")
    w2_d = nc.dram_tensor("W2", [HID, HID], BF16, kind="ExternalInput")
    b1_d = nc.dram_tensor("b1rep", [128, HID], F32, kind="ExternalInput")
    b2_d = nc.dram_tensor("b2rep", [128, HID], F32, kind="ExternalInput")

    h1_shard = nc.dram_tensor("h1_shard", [cfg.SHARD, HID], BF16)
    h1_full = nc.dram_tensor("h1_full", [N, HID], BF16, addr_space="Shared")
    out_d = nc.dram_tensor("pooled", [cfg.NSEGCH * 128, HID + 1], F32,
                           kind="ExternalOutput")
    if debug_taps:
        dbg_h1 = nc.dram_tensor("dbg_h1", [cfg.SHARD, HID], F32, kind="ExternalOutput")
        dbg_h2 = nc.dram_tensor("dbg_h2", [128, cfg.NBLK * HID], F32, kind="ExternalOutput")
        dbg_sc = nc.dram_tensor("dbg_sc", [128, 2 * cfg.NBLK], F32, kind="ExternalOutput")

    KIN = IN // 128   # k-chunks for W1 (2)

    with tile.TileContext(nc) as tc, ExitStack() as ctx:
        const = ctx.enter_context(tc.tile_pool(name="const", bufs=1))
        idxp = ctx.enter_context(tc.tile_pool(name="idxp", bufs=4))
        slotp = ctx.enter_context(tc.tile_pool(name="slotp", bufs=3))
        ebufp = ctx.enter_context(tc.tile_pool(name="ebufp", bufs=4))
        sp = ctx.enter_context(tc.tile_pool(name="sp", bufs=4))
        flshp = ctx.enter_context(tc.tile_pool(name="flshp", bufs=3))
        xtp = ctx.enter_context(tc.tile_pool(name="xtp", bufs=4))
        hp = ctx.enter_context(tc.tile_pool(name="hp", bufs=3))
        h2allp = ctx.enter_context(tc.tile_pool(name="h2allp", bufs=1))
        normp = ctx.enter_context(tc.tile_pool(name="normp", bufs=1))
        htp = ctx.enter_context(tc.tile_pool(name="htp", bufs=3))

        ctx_spmm = ctx.enter_context(ExitStack())
        ps_acc = ctx_spmm.enter_context(tc.tile_pool(name="ps_acc", bufs=4, space="PSUM"))
        ps_tr = ctx_spmm.enter_context(tc.tile_pool(name="ps_tr", bufs=1, space="PSUM"))
        ps_h = ctx_spmm.enter_context(tc.tile_pool(name="ps_h", bufs=2, space="PSUM"))

        # ---- constants ----
        iota128 = const.tile([128, 128], BF16)
        nc.sync.dma_start(iota128[:], iota_d[:])
        iotaseg = const.tile([128, cfg.NSEGCH * 128], F32)
        nc.sync.dma_start(iotaseg[:], iotas_d[:])
        ident = const.tile([128, 128], BF16)
        nc.sync.dma_start(ident[:], ident_d[:])
        segid = const.tile([128, cfg.NBLK], F32)
        nc.sync.dma_start(segid[:], segid_d[:])
        w1_sb = [const.tile([128, HID], BF16, tag=f"w1_{k}", name=f"w1_{k}")
                 for k in range(KIN)]
        for k in range(KIN):
            nc.sync.dma_start(w1_sb[k][:], w1_d[k * 128:(k + 1) * 128, :])
        w2_sb = const.tile([128, HID], BF16)
        nc.sync.dma_start(w2_sb[:], w2_d[:])
        b1_sb = const.tile([128, HID], F32)
        nc.sync.dma_start(b1_sb[:], b1_d[:])
        b2_sb = const.tile([128, HID], F32)
        nc.sync.dma_start(b2_sb[:], b2_d[:])

        h2_all = h2allp.tile([128, cfg.NBLK * HID], F32)
        norms2 = normp.tile([128, cfg.NBLK], F32)
        scale = normp.tile([128, cfg.NBLK], F32)
        na = normp.tile([128, cfg.NBLK], F32)
        nb_t = normp.tile([128, cfg.NBLK], F32)

        def spmm_layer(layer, table_ap, feat, out_block):
            """One spmm layer.  table_ap: DRAM [N, feat] gather table.
            out_block(g, b, agg_ps) consumes the accumulated [128(slot),
            feat] PSUM tile for global block nb=g*GRP+b.  One PSUM bank
            per block: start=True clears has_written bank-wide on HW, so
            accumulation groups must not share a bank."""
            for g in range(cfg.NGRP):
                accs = [ps_acc.tile([128, IN], F32, tag="acc", name=f"acc{b}")
                        for b in range(GRP)]

                def acc_slice(b):
                    return accs[b][:, :feat]

                for k in range(cfg.NCHUNK):
                    tiles_k = sched[g][k]
                    if not tiles_k:
                        continue
                    tbl = table_ap[k * cfg.CH:(k + 1) * cfg.CH, :]
                    # subcalls of <= SUB indices
                    for s0 in range(0, len(tiles_k), SUB // 128):
                        stiles = tiles_k[s0:s0 + SUB // 128]
                        nidx = len(stiles) * 128
                        col0 = stiles[0][0]  # global tile col
                        it = idxp.tile([128, SUB // 16], I16, tag="it")
                        nc.sync.dma_start(
                            it[:, :nidx // 16],
                            idx_d[:, col0 * 8:col0 * 8 + nidx // 16])
                        st = slotp.tile([128, SUB // 128], F32, tag="st")
                        nc.sync.dma_start(
                            st[:, :len(stiles)],
                            slot_d[:, col0:col0 + len(stiles)])
                        eb = ebufp.tile([128, (SUB // 128) * feat], BF16,
                                        tag=f"eb{layer}")
                        nc.gpsimd.dma_gather(
                            out_ap=eb[:, :len(stiles) * feat].rearrange(
                                "p (n f) -> p n f", f=feat),
                            in_ap=tbl,
                            idxs_ap=it[:, :nidx // 16],
                            num_idxs=nidx,
                            num_idxs_reg=nidx,
                            elem_size=feat,
                        )
                        for j, (tcol, b, st_f, sp_f) in enumerate(stiles):
                            s_t = sp.tile([128, 128], BF16, tag="s_t")
                            nc.vector.tensor_scalar(
                                s_t[:], iota128[:], st[:, j:j + 1], None,
                                ALU.is_equal)
                            nc.tensor.matmul(
                                acc_slice(b),
                                s_t[:],
                                eb[:, j * feat:(j + 1) * feat],
                                start=st_f, stop=sp_f,
                            )
                for b in range(GRP):
                    out_block(g, b, acc_slice(b))

        def l1_block(g, b, agg_ps):
            nb = g * GRP + b
            # copy PSUM f32 -> SBUF bf16
            ax = flshp.tile([128, IN], BF16, tag="ax1")
            nc.scalar.activation(ax[:], agg_ps, AF.Copy)
            h_ps = ps_h.tile([128, HID], F32, tag="hps", name="h_ps")
            for h in range(KIN):
                t_ps = ps_tr.tile([128, 128], BF16, tag="tps")
                nc.tensor.transpose(t_ps[:], ax[:, h * 128:(h + 1) * 128], ident[:])
                xt = xtp.tile([128, 128], BF16, tag="xt")
                nc.scalar.activation(xt[:], t_ps[:], AF.Copy)
                nc.tensor.matmul(h_ps[:], xt[:], w1_sb[h][:],
                                 start=(h == 0), stop=(h == KIN - 1))
            htmp = hp.tile([128, HID], F32, tag="htmp")
            nc.vector.tensor_add(htmp[:], h_ps[:], b1_sb[:])
            h1b = hp.tile([128, HID], BF16, tag="h1b")
            nc.scalar.activation(h1b[:], htmp[:], AF.Tanh)
            nc.sync.dma_start(h1_shard[nb * 128:(nb + 1) * 128, :], h1b[:])
            if debug_taps:
                h1f = hp.tile([128, HID], F32, tag="h1f")
                nc.scalar.activation(h1f[:], htmp[:], AF.Tanh)
                nc.sync.dma_start(dbg_h1[nb * 128:(nb + 1) * 128, :], h1f[:])

        def l2_block(g, b, agg_ps):
            nb = g * GRP + b
            a2 = flshp.tile([128, HID], BF16, tag="a22")
            nc.scalar.activation(a2[:], agg_ps, AF.Copy)
            t_ps = ps_tr.tile([128, 128], BF16, tag="tps")
            nc.tensor.transpose(t_ps[:], a2[:], ident[:])
            a2t = xtp.tile([128, 128], BF16, tag="xt")
            nc.scalar.activation(a2t[:], t_ps[:], AF.Copy)
            h_ps = ps_h.tile([128, HID], F32, tag="hps", name="h_ps")
            nc.tensor.matmul(h_ps[:], a2t[:], w2_sb[:], start=True, stop=True)
            htmp = hp.tile([128, HID], F32, tag="htmp")
            nc.vector.tensor_add(htmp[:], h_ps[:], b2_sb[:])
            nc.scalar.activation(h2_all[:, nb * HID:(nb + 1) * HID], htmp[:],
                                 AF.Tanh)

        # ---------------- layer 1 ----------------
        spmm_layer(1, x_d, IN, l1_block)

        # ---------------- exchange ----------------
        nc.gpsimd.collective_compute(
            "AllGather",
            ALU.bypass,
            ins=[h1_shard.ap().opt()],
            outs=[h1_full.ap().opt()],
            replica_groups=[list(range(cfg.NC))],
        )

        # ---------------- layer 2 ----------------
        spmm_layer(2, h1_full, HID, l2_block)

        # ---------------- norms + logmap scale ----------------
        for nbk in range(cfg.NBLK):
            h2b = h2_all[:, nbk * HID:(nbk + 1) * HID]
            sq = htp.tile([128, HID], F32, tag="sq")
            nc.vector.tensor_mul(sq[:], h2b, h2b)
            nc.vector.tensor_reduce(norms2[:, nbk:nbk + 1], sq[:],
                                    mybir.AxisListType.X, ALU.add)
        # norm = sqrt(max(ss, MIN_SS)); nclip = min(norm, MAXNORM)
        nc.vector.tensor_scalar_max(na[:], norms2[:], MIN_SS)
        nc.scalar.activation(nb_t[:], na[:], AF.Sqrt)        # nb_t = norm
        nc.vector.tensor_scalar_min(na[:], nb_t[:], MAXNORM)  # na = nclip
        # artanh(nclip) = 0.5*ln((1+n)/(1-n)); scale = artanh/norm
        one_m = normp.tile([128, cfg.NBLK], F32)
        nc.vector.tensor_scalar(one_m[:], na[:], -1.0, 1.0, ALU.mult, ALU.add)
        one_p = normp.tile([128, cfg.NBLK], F32)
        nc.vector.tensor_scalar_add(one_p[:], na[:], 1.0)
        rcp = normp.tile([128, cfg.NBLK], F32)
        nc.vector.reciprocal(rcp[:], one_m[:])
        rat = normp.tile([128, cfg.NBLK], F32)
        nc.vector.tensor_mul(rat[:], one_p[:], rcp[:])
        lg = normp.tile([128, cfg.NBLK], F32)
        nc.scalar.activation(lg[:], rat[:], AF.Ln)
        nc.vector.tensor_scalar_mul(lg[:], lg[:], 0.5)
        rcpn = normp.tile([128, cfg.NBLK], F32)
        nc.vector.reciprocal(rcpn[:], nb_t[:])
        nc.vector.tensor_mul(scale[:], lg[:], rcpn[:])

        if debug_taps:
            nc.sync.dma_start(dbg_h2[:], h2_all[:])
            nc.sync.dma_start(dbg_sc[:, :cfg.NBLK], norms2[:])
            nc.sync.dma_start(dbg_sc[:, cfg.NBLK:], scale[:])
        # ---------------- pooling ----------------
        ctx_spmm.close()
        ps_pool = ctx.enter_context(
            tc.tile_pool(name="ps_pool", bufs=max(cfg.NSEGCH, 1), space="PSUM"))
        pool_ps = [ps_pool.tile([128, HID + 1], F32, tag="pool", name=f"pool{sc}")
                   for sc in range(cfg.NSEGCH)]
        for nbk in range(cfg.NBLK):
            h2b = h2_all[:, nbk * HID:(nbk + 1) * HID]
            ht = htp.tile([128, HID + 1], BF16, tag="ht")
            nc.vector.tensor_scalar(ht[:, :HID], h2b, scale[:, nbk:nbk + 1],
                                    None, ALU.mult)
            nc.vector.memset(ht[:, HID:HID + 1], 1.0)
            for sc in range(cfg.NSEGCH):
                sg = sp.tile([128, 128], BF16, tag="sg")
                nc.vector.tensor_scalar(
                    sg[:], iotaseg[:, sc * 128:(sc + 1) * 128],
                    segid[:, nbk:nbk + 1], None, ALU.is_equal)
                nc.tensor.matmul(
                    pool_ps[sc][:], sg[:], ht[:],
                    start=(nbk == 0), stop=(nbk == cfg.NBLK - 1))
        for sc in range(cfg.NSEGCH):
            po = htp.tile([128, HID + 1], F32, tag="po")
            nc.vector.tensor_copy(po[:], pool_ps[sc][:])
            nc.sync.dma_start(out_d[sc * 128:(sc + 1) * 128, :], po[:])

    nc.compile()
    return nc


def host_inputs(cfg, x, seg_ids, W1, b1, W2, b2, per_core):
    """Per-core in_maps for run_bass_kernel_spmd."""
    N, IN, HID = cfg.N, cfg.IN, cfg.HID
    x_bf16 = np.ascontiguousarray(x.astype(ml_dtypes.bfloat16))
    iota128 = np.tile(np.arange(128, dtype=np.float32), (128, 1)).astype(ml_dtypes.bfloat16)
    iotaseg = np.tile(np.arange(cfg.NSEGCH * 128, dtype=np.float32), (128, 1))
    ident = np.eye(128, dtype=np.float32).astype(ml_dtypes.bfloat16)
    w1 = np.ascontiguousarray(W1.astype(ml_dtypes.bfloat16))
    w2 = np.ascontiguousarray(W2.astype(ml_dtypes.bfloat16))
    b1r = np.tile(np.asarray(b1, np.float32), (128, 1))
    b2r = np.tile(np.asarray(b2, np.float32), (128, 1))
    seg = np.asarray(seg_ids, np.float32)
    maps = []
    for c in range(cfg.NC):
        segc = seg[c * cfg.SHARD:(c + 1) * cfg.SHARD].reshape(cfg.NBLK, 128).T
        maps.append({
            "x_bf16": x_bf16,
            "idx16": per_core[c]["idx16"],
            "dstslot": per_core[c]["dstslot"],
            "segid": np.ascontiguousarray(segc),
            "iota128": iota128,
            "iota_seg": np.ascontiguousarray(iotaseg.astype(np.float32)),
            "ident": ident,
            "W1": w1,
            "W2": w2,
            "b1rep": b1r,
            "b2rep": b2r,
        })
    return maps


def host_epilogue(cfg, partials, batch_size, max_comments):
    """partials: list of per-core [NSEGCH*128, HID+1] f32."""
    acc = np.zeros_like(partials[0], dtype=np.float64)
    for p in partials:
        acc += p.astype(np.float64)
    acc = acc.astype(np.float32)
    nseg = cfg.NSEG
    sums = acc[:nseg, :cfg.HID]
    counts = acc[:nseg, cfg.HID]
    agg = sums / np.maximum(counts, 1.0)[:, None]
    # expmap0 then proj
    ss = np.maximum(np.sum(agg * agg, axis=1), MIN_SS).astype(np.float32)
    norm = np.sqrt(ss)
    y = agg * (np.tanh(norm) / norm)[:, None]
    ssy = np.maximum(np.sum(y * y, axis=1), MIN_SS).astype(np.float32)
    ny = np.sqrt(ssy)
    f = np.where(ny > MAXNORM, MAXNORM / ny, 1.0).astype(np.float32)
    y = y * f[:, None]
    return y.reshape(int(batch_size), int(max_comments), cfg.HID)


# ---------------- numpy reference (for arbitrary sizes) ----------------

def np_reference(x, src, dst, seg_ids, W1, b1, W2, b2, batch_size, max_comments):
    n = x.shape[0]

    def seg_sum(vals, ids, nseg):
        out = np.zeros((nseg, vals.shape[1]), np.float32)
        np.add.at(out, ids, vals)
        return out

    def rownorm(v):
        return np.sqrt(np.maximum(np.sum(v * v, axis=1, keepdims=True), MIN_SS))

    def proj(v):
        nn = rownorm(v)
        return np.where(nn > MAXNORM, v / nn * MAXNORM, v)

    def logmap0(v):
        nn = rownorm(v)
        arg = np.minimum(nn, 1 - 1e-7)
        return v * np.arctanh(arg) / nn

    def expmap0(v):
        nn = rownorm(v)
        return v * np.tanh(nn) / nn

    h = np.tanh(seg_sum(x[src] @ W1, dst, n) + b1)
    h = np.tanh(seg_sum(h[src] @ W2, dst, n) + b2)
    h = logmap0(proj(h))
    nseg = int(batch_size) * int(max_comments)
    sums = seg_sum(h, seg_ids, nseg)
    counts = np.zeros(nseg, np.float32)
    np.add.at(counts, seg_ids, 1.0)
    agg = sums / np.maximum(counts, 1.0)[:, None]
    agg = proj(expmap0(agg))
    return agg.reshape(int(batch_size), int(max_comments), -1)


# ====================================================================
# Harness entry point: kernel(**inputs) -> np.ndarray
# ====================================================================

_CACHE = {}


def kernel(x, src, dst, seg_ids, W1, b1, W2, b2, batch_size, max_comments):
    """Full-input GNN ComEnc kernel on 8 Trainium2 NeuronCores.

    Accepts the unsharded inputs of reference.setup_inputs() and returns
    the full (batch, max_comments, HID) float32 output.
    """
    from concourse.bass_utils import run_bass_kernel_spmd

    x = np.asarray(x, dtype=np.float32)
    src = np.asarray(src).astype(np.int64)
    dst = np.asarray(dst).astype(np.int64)
    seg_ids = np.asarray(seg_ids).astype(np.int64)
    W1 = np.asarray(W1, dtype=np.float32)
    b1 = np.asarray(b1, dtype=np.float32)
    W2 = np.asarray(W2, dtype=np.float32)
    b2 = np.asarray(b2, dtype=np.float32)
    bs = int(np.asarray(batch_size))
    mc = int(np.asarray(max_comments))

    n_nodes, in_dim = x.shape
    hid = W1.shape[1]
    nseg = bs * mc
    n_cores = 8

    cfg = Cfg(n_nodes, in_dim, hid, nseg, n_cores)
    ntiles, per_core = host_prep(cfg, src, dst)

    key = (n_nodes, in_dim, hid, nseg, ntiles.tobytes())
    if key in _CACHE:
        nc = _CACHE[key]
    else:
        nc = build(cfg, ntiles)
        _CACHE.clear()
        _CACHE[key] = nc

    maps = host_inputs(cfg, x, seg_ids, W1, b1, W2, b2, per_core)
    res = run_bass_kernel_spmd(nc, maps, core_ids=list(range(n_cores)))
    partials = [r["pooled"] for r in res.results]
    out = host_epilogue(cfg, partials, bs, mc)
    return np.ascontiguousarray(out.astype(np.float32))



# revision 8
# speedup vs baseline: 1.5060x; 1.5060x over previous
"""GNN message-passing kernel for Trainium2 (8 NeuronCores, SPMD).

Computation (see np_reference):
  h1 = tanh((A x) @ W1 + b1)      [A = raw adjacency, segsum over dst]
  h2 = tanh((A h1) @ W2 + b2)
  ht = logmap0(proj(h2))          (rowwise scale)
  pooled[seg] = sum over nodes; counts + expmap on host.

Sharding: nodes split contiguously over cores (dst-shard).  The spmm is a
one-hot matmul per 128-edge tile; gathered rows come from gpsimd.dma_gather
with int16 indices (tables chunked to 32768 rows).

v2 layout:
 - one dma_gather per (group, chunk) cell; idx/slot staged per group in one
   DMA each (ACT + SP queues); h1 stored per group.
 - the h1 AllGather is split in two: AG_A (shard rows 0..8191) fires after
   the first 16 L1 groups and overlaps the last 16; AG_B overlaps L2
   phase A (chunks 0,1 read from h1_fullA).  Layer-2 accumulation is split
   per phase with bf16 partials in SBUF.
 - layer 2 accumulates transposed (acc[f, slot]) so its per-block PE
   transpose disappears; pooling is out[hid, seg] with one [128, NSEG128]
   one-hot per block into a single PSUM bank; counts computed on host.
"""

import math
from contextlib import ExitStack

import numpy as np
import ml_dtypes

import concourse.bass as bass
import concourse.tile as tile
import concourse.bacc as bacc
from concourse import mybir

BF16 = mybir.dt.bfloat16
F32 = mybir.dt.float32
I16 = mybir.dt.int16
AF = mybir.ActivationFunctionType
ALU = mybir.AluOpType

MAXNORM = 1.0 - 1e-5
MIN_SS = 1e-15

GRP = 4             # dst blocks (of 128 nodes) per PSUM group
CAP_TILES = 8       # max 128-edge tiles per dma_gather call


class Cfg:
    def __init__(self, n_nodes, in_dim, hid, n_seg, n_cores):
        self.N = n_nodes
        self.IN = in_dim
        self.HID = hid
        self.NSEG = n_seg
        self.NC = n_cores
        self.SHARD = n_nodes // n_cores
        assert self.SHARD % 128 == 0
        self.NBLK = self.SHARD // 128
        assert self.NBLK % GRP == 0
        self.NGRP = self.NBLK // GRP
        self.CH = min(32768, n_nodes)
        assert n_nodes % self.CH == 0
        self.NCHUNK = n_nodes // self.CH          # 4
        self.NSEGCH = (n_seg + 127) // 128        # 3
        self.HALF = self.SHARD // 2               # 8192 rows per AG piece


def _edge_fields(cfg, src, dst, permute_src):
    """Per-edge core/block/slot + chunk/idx (optionally with the AG-split
    permutation applied to the src->table-row mapping)."""
    core = dst // cfg.SHARD
    blk = (dst % cfg.SHARD) // 128
    slot = dst % 128
    if permute_src:
        # table row = (srchalf)*N/2 + srccore*HALF + offset
        half = (src % cfg.SHARD) // cfg.HALF
        row = half * (cfg.N // 2) + (src // cfg.SHARD) * cfg.HALF + (src % cfg.HALF)
    else:
        row = src
    chunk = row // cfg.CH
    idx = row % cfg.CH
    return core, blk, slot, chunk, idx


def _layout_layer(cfg, core, blk, slot, chunk, idx, cell_order, force_min):
    """Generic canonical edge-stream builder.

    cell_order: list of (g, k) in processing order (cells iterate b inner).
    force_min: list of lists of k-sets; for each (g, b), each k-set must own
      >= 1 tile (padding goes to the set's first k).
    Returns (ntiles[NGRP, NCHUNK, GRP], per-core idx16 [128, TOT/16] int16,
             per-core slots [128, NTILES] f32, sched).
    """
    NC, NGRP, NCHUNK = cfg.NC, cfg.NGRP, cfg.NCHUNK
    g_all = blk // GRP
    b_all = blk % GRP

    counts = np.zeros((NC, NGRP, NCHUNK, GRP), dtype=np.int64)
    np.add.at(counts, (core, g_all, chunk, b_all), 1)
    mx = counts.max(axis=0)
    ntiles = (mx + 127) // 128
    # force_min: ensure each (g, b) has >=1 tile within each k-set
    for kset in force_min:
        ks = list(kset)
        empty = ntiles[:, ks, :].sum(axis=1) == 0     # [NGRP, GRP]
        sub = ntiles[:, ks[0], :]
        sub[empty] = 1
        ntiles[:, ks[0], :] = sub

    # cell rank in processing order
    cell_rank = np.full((NGRP, NCHUNK, GRP), -1, dtype=np.int64)
    rank = 0
    cells = []      # (g, k, b) in rank order
    for (g, k) in cell_order:
        for b in range(GRP):
            cell_rank[g, k, b] = rank
            cells.append((g, k, b))
            rank += 1
    ncells = rank
    nt_flat = np.zeros(ncells, dtype=np.int64)
    for r, (g, k, b) in enumerate(cells):
        nt_flat[r] = ntiles[g, k, b]
    base = np.zeros(ncells + 1, dtype=np.int64)
    np.cumsum(nt_flat * 128, out=base[1:])
    NTILES = int(nt_flat.sum())
    TOT = NTILES * 128

    edge_rank = cell_rank[g_all, chunk, b_all]
    idx16_cores = []
    slot_cores = []
    for c in range(NC):
        sel = np.nonzero(core == c)[0]
        r = edge_rank[sel]
        order = np.argsort(r, kind="stable")
        sel = sel[order]
        r = r[order]
        # position within cell
        cnt = np.bincount(r, minlength=ncells)
        first = np.zeros(ncells, dtype=np.int64)
        np.cumsum(cnt[:-1], out=first[1:])
        within = np.arange(len(sel)) - first[r]
        pos = base[r] + within
        idx16 = np.zeros(TOT, dtype=np.int16)
        slots = np.full(TOT, -1.0, dtype=np.float32)
        idx16[pos] = idx[sel]
        slots[pos] = slot[sel]
        iw = idx16.reshape(-1, 16).T                  # [16, TOT/16]
        iw = np.tile(iw, (8, 1)).copy()               # [128, TOT/16]
        sl = slots.reshape(NTILES, 128).T.copy()      # [128, NTILES]
        idx16_cores.append(iw.astype(np.int16))
        slot_cores.append(sl.astype(np.float32))

    # schedule: per (g, k) in order -> list of (tcol, b, start, stop)
    # start/stop scope: for each force_min k-set (accumulation scope)
    scope_of_k = {}
    for si, kset in enumerate(force_min):
        for k in kset:
            scope_of_k[k] = si
    tot_b = np.zeros((NGRP, len(force_min), GRP), dtype=np.int64)
    for si, kset in enumerate(force_min):
        for k in kset:
            tot_b[:, si, :] += ntiles[:, k, :]
    seen = np.zeros((NGRP, len(force_min), GRP), dtype=np.int64)
    sched = {}
    tcol = 0
    for (g, k) in cell_order:
        si = scope_of_k[k]
        tiles = []
        for b in range(GRP):
            for _ in range(int(ntiles[g, k, b])):
                st = seen[g, si, b] == 0
                sp = seen[g, si, b] == tot_b[g, si, b] - 1
                tiles.append((tcol, b, bool(st), bool(sp)))
                seen[g, si, b] += 1
                tcol += 1
        sched[(g, k)] = tiles
    return ntiles, idx16_cores, slot_cores, sched, NTILES


def host_prep(cfg, src, dst):
    src = np.asarray(src).astype(np.int64)
    dst = np.asarray(dst).astype(np.int64)
    # layer 1: natural src chunking, k inner per group, one accum scope
    c1 = _edge_fields(cfg, src, dst, permute_src=False)
    order1 = [(g, k) for g in range(cfg.NGRP) for k in range(cfg.NCHUNK)]
    l1 = _layout_layer(cfg, *c1, order1, [range(cfg.NCHUNK)])
    # layer 2: permuted table rows; phase A (k=0,1) then phase B (k=2,3)
    c2 = _edge_fields(cfg, src, dst, permute_src=True)
    order2 = ([(g, k) for g in range(cfg.NGRP) for k in (0, 1)] +
              [(g, k) for g in range(cfg.NGRP) for k in (2, 3)])
    l2 = _layout_layer(cfg, *c2, order2, [(0, 1), (2, 3)])
    return l1, l2


def build(cfg, l1, l2):
    N, IN, HID = cfg.N, cfg.IN, cfg.HID
    NGRP, NCHUNK, NBLK = cfg.NGRP, cfg.NCHUNK, cfg.NBLK
    ntiles1, _, _, sched1, NT1 = l1
    ntiles2, _, _, sched2, NT2 = l2
    NSEGC = cfg.NSEGCH * 128                       # 384
    KIN = IN // 128

    nc = bacc.Bacc("TRN2", target_bir_lowering=False)

    x_d = nc.dram_tensor("x_bf16", [N, IN], BF16, kind="ExternalInput")
    idx1_d = nc.dram_tensor("idx1", [128, NT1 * 8], I16, kind="ExternalInput")
    slot1_d = nc.dram_tensor("slot1", [128, NT1], F32, kind="ExternalInput")
    idx2_d = nc.dram_tensor("idx2", [128, NT2 * 8], I16, kind="ExternalInput")
    slot2_d = nc.dram_tensor("slot2", [128, NT2], F32, kind="ExternalInput")
    segid_d = nc.dram_tensor("segid", [128, NBLK], F32, kind="ExternalInput")
    iota_d = nc.dram_tensor("iota128", [128, 128], BF16, kind="ExternalInput")
    iotas_d = nc.dram_tensor("iota_seg", [128, NSEGC], F32, kind="ExternalInput")
    ident_d = nc.dram_tensor("ident", [128, 128], BF16, kind="ExternalInput")
    w1_d = nc.dram_tensor("W1", [IN, HID], BF16, kind="ExternalInput")
    w2_d = nc.dram_tensor("W2", [HID, HID], BF16, kind="ExternalInput")
    b1_d = nc.dram_tensor("b1rep", [128, HID], F32, kind="ExternalInput")
    b2_d = nc.dram_tensor("b2rep", [128, HID], F32, kind="ExternalInput")

    h1_shard = nc.dram_tensor("h1_shard", [cfg.SHARD, HID], BF16)
    h1_fullA = nc.dram_tensor("h1_fullA", [N // 2, HID], BF16, addr_space="Shared")
    h1_fullB = nc.dram_tensor("h1_fullB", [N // 2, HID], BF16, addr_space="Shared")
    out_d = nc.dram_tensor("pooled", [128, NSEGC], F32, kind="ExternalOutput")

    max_cell1 = int(ntiles1.sum(axis=2).max())
    max_cell2 = int(ntiles2.sum(axis=2).max())
    grp_tiles1 = int(ntiles1.sum(axis=(1, 2)).max())
    grp_tiles2 = int(ntiles2.sum(axis=(1, 2)).max())

    with tile.TileContext(nc) as tc, ExitStack() as ctx:
        const = ctx.enter_context(tc.tile_pool(name="const", bufs=1))
        idxp = ctx.enter_context(tc.tile_pool(name="idxp", bufs=3))
        slotp = ctx.enter_context(tc.tile_pool(name="slotp", bufs=3))
        ebufp = ctx.enter_context(tc.tile_pool(name="ebufp", bufs=4))
        sp = ctx.enter_context(tc.tile_pool(name="sp", bufs=4))
        flshp = ctx.enter_context(tc.tile_pool(name="flshp", bufs=3))
        xtp = ctx.enter_context(tc.tile_pool(name="xtp", bufs=3))
        hp = ctx.enter_context(tc.tile_pool(name="hp", bufs=4))
        h2p = ctx.enter_context(tc.tile_pool(name="h2p", bufs=6))
        partp = ctx.enter_context(tc.tile_pool(name="partp", bufs=1))
        normp = ctx.enter_context(tc.tile_pool(name="normp", bufs=4))
        htp = ctx.enter_context(tc.tile_pool(name="htp", bufs=3))
        h1gp = ctx.enter_context(tc.tile_pool(name="h1gp", bufs=2))

        ps_acc = ctx.enter_context(tc.tile_pool(name="ps_acc", bufs=GRP, space="PSUM"))
        ps_tr = ctx.enter_context(tc.tile_pool(name="ps_tr", bufs=1, space="PSUM"))
        ps_h = ctx.enter_context(tc.tile_pool(name="ps_h", bufs=2, space="PSUM"))
        ps_pool = ctx.enter_context(tc.tile_pool(name="ps_pool", bufs=1, space="PSUM"))

        # ---- constants ----
        iota128 = const.tile([128, 128], BF16)
        nc.sync.dma_start(iota128[:], iota_d[:])
        iotaseg = const.tile([128, NSEGC], F32)
        nc.sync.dma_start(iotaseg[:], iotas_d[:])
        ident = const.tile([128, 128], BF16)
        nc.sync.dma_start(ident[:], ident_d[:])
        segid = const.tile([128, NBLK], F32)
        nc.sync.dma_start(segid[:], segid_d[:])
        w1_sb = [const.tile([128, HID], BF16, tag=f"w1_{k}", name=f"w1_{k}")
                 for k in range(KIN)]
        for k in range(KIN):
            nc.sync.dma_start(w1_sb[k][:], w1_d[k * 128:(k + 1) * 128, :])
        w2_sb = const.tile([128, HID], BF16)
        nc.sync.dma_start(w2_sb[:], w2_d[:])
        b1_sb = const.tile([128, HID], F32)
        nc.sync.dma_start(b1_sb[:], b1_d[:])
        b2_sb = const.tile([128, HID], F32)
        nc.sync.dma_start(b2_sb[:], b2_d[:])

        partA = partp.tile([128, NBLK * HID], BF16)
        pool_ps = ps_pool.tile([128, NSEGC], F32, name="pool_ps")

        def run_group(g, ks, sched, idx_d, slot_d, tables, feat, flip,
                      out_block, grp_cols, cell_cap):
            """Process cells (g, k in ks); call out_block(g, b, acc) for
            completed accumulations (only when the scope's stop fired)."""
            tiles_all = [t for k in ks for t in sched[(g, k)]]
            if not tiles_all:
                return
            tcol0 = tiles_all[0][0]
            ntg = len(tiles_all)
            it = idxp.tile([128, grp_cols * 8], I16, tag="it")
            nc.scalar.dma_start(it[:, :ntg * 8],
                                idx_d[:, tcol0 * 8:(tcol0 + ntg) * 8])
            st = slotp.tile([128, grp_cols], F32, tag="st")
            nc.sync.dma_start(st[:, :ntg], slot_d[:, tcol0:tcol0 + ntg])

            accs = {}
            done = []
            for k in ks:
                tiles_k = sched[(g, k)]
                if not tiles_k:
                    continue
                tbl = tables[k]
                for s0 in range(0, len(tiles_k), CAP_TILES):
                    stiles = tiles_k[s0:s0 + CAP_TILES]
                    nidx = len(stiles) * 128
                    c0 = stiles[0][0]
                    eb = ebufp.tile([128, cell_cap * feat], BF16,
                                    tag=f"eb{feat}")
                    nc.gpsimd.dma_gather(
                        out_ap=eb[:, :len(stiles) * feat].rearrange(
                            "p (n f) -> p n f", f=feat),
                        in_ap=tbl,
                        idxs_ap=it[:, (c0 - tcol0) * 8:(c0 - tcol0) * 8 + nidx // 16],
                        num_idxs=nidx,
                        num_idxs_reg=nidx,
                        elem_size=feat,
                    )
                    for j, (tcol, b, st_f, sp_f) in enumerate(stiles):
                        if b not in accs:
                            accs[b] = ps_acc.tile([128, IN], F32, tag="acc",
                                                  name=f"acc{b}")
                        s_t = sp.tile([128, 128], BF16, tag="s_t")
                        nc.vector.tensor_scalar(
                            s_t[:], iota128[:], st[:, tcol - tcol0:tcol - tcol0 + 1],
                            None, ALU.is_equal)
                        if flip:
                            nc.tensor.matmul(
                                accs[b][:, :feat],
                                eb[:, j * feat:(j + 1) * feat], s_t[:],
                                start=st_f, stop=sp_f)
                        else:
                            nc.tensor.matmul(
                                accs[b][:, :feat],
                                s_t[:], eb[:, j * feat:(j + 1) * feat],
                                start=st_f, stop=sp_f)
                        if sp_f:
                            done.append(b)
            for b in done:
                out_block(g, b, accs[b][:, :feat])

        # ================= layer 1 =================
        x_tables = [x_d[k * cfg.CH:(k + 1) * cfg.CH, :] for k in range(NCHUNK)]

        h1g = [None]

        def l1_block(g, b, agg_ps):
            nb = g * GRP + b
            ax = flshp.tile([128, IN], BF16, tag="ax1")
            nc.scalar.activation(ax[:], agg_ps, AF.Copy)
            h_ps = ps_h.tile([128, HID], F32, tag="hps", name="h_ps")
            for h in range(KIN):
                t_ps = ps_tr.tile([128, 128], BF16, tag="tps")
                nc.tensor.transpose(t_ps[:], ax[:, h * 128:(h + 1) * 128], ident[:])
                xt = xtp.tile([128, 128], BF16, tag="xt")
                nc.scalar.activation(xt[:], t_ps[:], AF.Copy)
                nc.tensor.matmul(h_ps[:], xt[:], w1_sb[h][:],
                                 start=(h == 0), stop=(h == KIN - 1))
            htmp = hp.tile([128, HID], F32, tag="htmp")
            nc.vector.tensor_add(htmp[:], h_ps[:], b1_sb[:])
            nc.scalar.activation(h1g[0][:, b * HID:(b + 1) * HID], htmp[:], AF.Tanh)

        for g in range(NGRP):
            h1g[0] = h1gp.tile([128, GRP * HID], BF16, tag="h1g", name="h1g")
            run_group(g, range(NCHUNK), sched1, idx1_d, slot1_d, x_tables,
                      IN, False, l1_block, grp_tiles1,
                      min(max_cell1, CAP_TILES))
            nc.sync.dma_start(
                h1_shard[g * GRP * 128:(g + 1) * GRP * 128, :].rearrange(
                    "(b s) h -> s b h", s=128),
                h1g[0][:].rearrange("s (b h) -> s b h", h=HID))
            if g == NGRP // 2 - 1:
                nc.gpsimd.collective_compute(
                    "AllGather", ALU.bypass,
                    ins=[h1_shard[0:cfg.HALF, :].opt()],
                    outs=[h1_fullA.ap().opt()],
                    replica_groups=[list(range(cfg.NC))])
        nc.gpsimd.collective_compute(
            "AllGather", ALU.bypass,
            ins=[h1_shard[cfg.HALF:cfg.SHARD, :].opt()],
            outs=[h1_fullB.ap().opt()],
            replica_groups=[list(range(cfg.NC))])

        # ================= layer 2 =================
        h_tables = {0: h1_fullA[0:cfg.CH, :], 1: h1_fullA[cfg.CH:2 * cfg.CH, :],
                    2: h1_fullB[0:cfg.CH, :], 3: h1_fullB[cfg.CH:2 * cfg.CH, :]}

        def l2a_block(g, b, agg_ps):
            nb = g * GRP + b
            nc.scalar.activation(partA[:, nb * HID:(nb + 1) * HID], agg_ps,
                                 AF.Copy)

        for g in range(NGRP):
            run_group(g, (0, 1), sched2, idx2_d, slot2_d, h_tables,
                      HID, True, l2a_block, grp_tiles2,
                      min(max_cell2, CAP_TILES))

        # phase B + norms + pooling interleaved per group
        ss_g = [None]
        sc_g = [None]
        h2_tiles = {}

        def l2b_block(g, b, agg_ps):
            nb = g * GRP + b
            a2t = flshp.tile([128, HID], BF16, tag="a2t")
            nc.vector.tensor_add(a2t[:], agg_ps,
                                 partA[:, nb * HID:(nb + 1) * HID])
            h_ps = ps_h.tile([128, HID], F32, tag="hps", name="h_ps")
            nc.tensor.matmul(h_ps[:], a2t[:], w2_sb[:], start=True, stop=True)
            htmp = hp.tile([128, HID], F32, tag="htmp")
            nc.vector.tensor_add(htmp[:], h_ps[:], b2_sb[:])
            h2b = h2p.tile([128, HID], F32, tag="h2b")
            nc.scalar.activation(h2b[:], htmp[:], AF.Tanh)
            h2_tiles[b] = h2b
            sq = hp.tile([128, HID], F32, tag="sq")
            nc.vector.tensor_mul(sq[:], h2b[:], h2b[:])
            nc.vector.tensor_reduce(ss_g[0][:, b:b + 1], sq[:],
                                    mybir.AxisListType.X, ALU.add)

        for g in range(NGRP):
            ss_g[0] = normp.tile([128, GRP], F32, tag="ss", name="ss")
            sc_g[0] = normp.tile([128, GRP], F32, tag="sc", name="sc")
            h2_tiles.clear()
            run_group(g, (2, 3), sched2, idx2_d, slot2_d, h_tables,
                      HID, True, l2b_block, grp_tiles2,
                      min(max_cell2, CAP_TILES))
            # scale = artanh(min(sqrt(max(ss,MIN)), MAXNORM)) / sqrt(...)
            ss, sc = ss_g[0], sc_g[0]
            na = normp.tile([128, GRP], F32, tag="na")
            nb_t = normp.tile([128, GRP], F32, tag="nb")
            nc.vector.tensor_scalar_max(na[:], ss[:], MIN_SS)
            nc.scalar.activation(nb_t[:], na[:], AF.Sqrt)
            nc.vector.tensor_scalar_min(na[:], nb_t[:], MAXNORM)
            one_m = normp.tile([128, GRP], F32, tag="om")
            nc.vector.tensor_scalar(one_m[:], na[:], -1.0, 1.0, ALU.mult, ALU.add)
            one_p = normp.tile([128, GRP], F32, tag="op")
            nc.vector.tensor_scalar_add(one_p[:], na[:], 1.0)
            rcp = normp.tile([128, GRP], F32, tag="rc")
            nc.vector.reciprocal(rcp[:], one_m[:])
            rat = normp.tile([128, GRP], F32, tag="ra")
            nc.vector.tensor_mul(rat[:], one_p[:], rcp[:])
            lg = normp.tile([128, GRP], F32, tag="lg")
            nc.scalar.activation(lg[:], rat[:], AF.Ln)
            rcpn = normp.tile([128, GRP], F32, tag="rn")
            nc.vector.reciprocal(rcpn[:], nb_t[:])
            nc.vector.tensor_mul(rcpn[:], lg[:], rcpn[:])
            nc.vector.tensor_scalar_mul(sc[:], rcpn[:], 0.5)
            for b in range(GRP):
                nb = g * GRP + b
                ht = htp.tile([128, HID], BF16, tag="ht")
                nc.vector.tensor_scalar(ht[:], h2_tiles[b][:], sc[:, b:b + 1],
                                        None, ALU.mult)
                sg = sp.tile([128, NSEGC], BF16, tag="sg")
                nc.vector.tensor_scalar(sg[:], iotaseg[:],
                                        segid[:, nb:nb + 1], None, ALU.is_equal)
                nc.tensor.matmul(pool_ps[:], ht[:], sg[:],
                                 start=(nb == 0), stop=(nb == NBLK - 1))

        po = htp.tile([128, NSEGC], F32, tag="po")
        nc.vector.tensor_copy(po[:], pool_ps[:])
        nc.sync.dma_start(out_d[:], po[:])

    nc.compile()
    return nc


def host_inputs(cfg, x, seg_ids, W1, b1, W2, b2, l1, l2):
    N, IN, HID = cfg.N, cfg.IN, cfg.HID
    _, idx1_cores, slot1_cores, _, _ = l1
    _, idx2_cores, slot2_cores, _, _ = l2
    x_bf16 = np.ascontiguousarray(x.astype(ml_dtypes.bfloat16))
    iota128 = np.tile(np.arange(128, dtype=np.float32), (128, 1)).astype(ml_dtypes.bfloat16)
    iotaseg = np.tile(np.arange(cfg.NSEGCH * 128, dtype=np.float32), (128, 1)).astype(np.float32)
    ident = np.eye(128, dtype=np.float32).astype(ml_dtypes.bfloat16)
    w1 = np.ascontiguousarray(W1.astype(ml_dtypes.bfloat16))
    w2 = np.ascontiguousarray(W2.astype(ml_dtypes.bfloat16))
    b1r = np.tile(np.asarray(b1, np.float32), (128, 1))
    b2r = np.tile(np.asarray(b2, np.float32), (128, 1))
    seg = np.asarray(seg_ids, np.float32)
    maps = []
    for c in range(cfg.NC):
        segc = seg[c * cfg.SHARD:(c + 1) * cfg.SHARD].reshape(cfg.NBLK, 128).T
        maps.append({
            "x_bf16": x_bf16,
            "idx1": idx1_cores[c], "slot1": slot1_cores[c],
            "idx2": idx2_cores[c], "slot2": slot2_cores[c],
            "segid": np.ascontiguousarray(segc),
            "iota128": iota128,
            "iota_seg": np.ascontiguousarray(iotaseg),
            "ident": ident,
            "W1": w1, "W2": w2, "b1rep": b1r, "b2rep": b2r,
        })
    return maps


def host_epilogue(cfg, partials, seg_ids, batch_size, max_comments):
    """partials: list of per-core [128, NSEGC] f32 (hid x seg)."""
    acc = np.zeros_like(partials[0], dtype=np.float64)
    for p in partials:
        acc += p.astype(np.float64)
    sums = acc.T[:cfg.NSEG, :].astype(np.float32)        # [NSEG, HID]
    counts = np.bincount(np.asarray(seg_ids).astype(np.int64),
                         minlength=cfg.NSEG).astype(np.float32)
    agg = sums / np.maximum(counts, 1.0)[:, None]
    ss = np.maximum(np.sum(agg * agg, axis=1), MIN_SS).astype(np.float32)
    norm = np.sqrt(ss)
    y = agg * (np.tanh(norm) / norm)[:, None]
    ssy = np.maximum(np.sum(y * y, axis=1), MIN_SS).astype(np.float32)
    ny = np.sqrt(ssy)
    f = np.where(ny > MAXNORM, MAXNORM / ny, 1.0).astype(np.float32)
    y = y * f[:, None]
    return y.reshape(int(batch_size), int(max_comments), cfg.HID)


# ---------------- numpy reference (for arbitrary sizes) ----------------

def np_reference(x, src, dst, seg_ids, W1, b1, W2, b2, batch_size, max_comments):
    n = x.shape[0]

    def seg_sum(vals, ids, nseg):
        out = np.zeros((nseg, vals.shape[1]), np.float32)
        np.add.at(out, ids, vals)
        return out

    def rownorm(v):
        return np.sqrt(np.maximum(np.sum(v * v, axis=1, keepdims=True), MIN_SS))

    def proj(v):
        nn = rownorm(v)
        return np.where(nn > MAXNORM, v / nn * MAXNORM, v)

    def logmap0(v):
        nn = rownorm(v)
        arg = np.minimum(nn, 1 - 1e-7)
        return v * np.arctanh(arg) / nn

    def expmap0(v):
        nn = rownorm(v)
        return v * np.tanh(nn) / nn

    h = np.tanh(seg_sum(x[src] @ W1, dst, n) + b1)
    h = np.tanh(seg_sum(h[src] @ W2, dst, n) + b2)
    h = logmap0(proj(h))
    nseg = int(batch_size) * int(max_comments)
    sums = seg_sum(h, seg_ids, nseg)
    counts = np.zeros(nseg, np.float32)
    np.add.at(counts, seg_ids, 1.0)
    agg = sums / np.maximum(counts, 1.0)[:, None]
    agg = proj(expmap0(agg))
    return agg.reshape(int(batch_size), int(max_comments), -1)


# ====================================================================
# Harness entry point: kernel(**inputs) -> np.ndarray
# ====================================================================

_CACHE = {}


def kernel(x, src, dst, seg_ids, W1, b1, W2, b2, batch_size, max_comments):
    """Full-input GNN ComEnc kernel on 8 Trainium2 NeuronCores."""
    from concourse.bass_utils import run_bass_kernel_spmd

    x = np.asarray(x, dtype=np.float32)
    src = np.asarray(src).astype(np.int64)
    dst = np.asarray(dst).astype(np.int64)
    seg_ids = np.asarray(seg_ids).astype(np.int64)
    W1 = np.asarray(W1, dtype=np.float32)
    b1 = np.asarray(b1, dtype=np.float32)
    W2 = np.asarray(W2, dtype=np.float32)
    b2 = np.asarray(b2, dtype=np.float32)
    bs = int(np.asarray(batch_size))
    mc = int(np.asarray(max_comments))

    n_nodes, in_dim = x.shape
    hid = W1.shape[1]
    nseg = bs * mc
    n_cores = 8

    cfg = Cfg(n_nodes, in_dim, hid, nseg, n_cores)
    l1, l2 = host_prep(cfg, src, dst)

    key = (n_nodes, in_dim, hid, nseg, l1[0].tobytes(), l2[0].tobytes())
    if key in _CACHE:
        nc = _CACHE[key]
    else:
        nc = build(cfg, l1, l2)
        _CACHE.clear()
        _CACHE[key] = nc

    maps = host_inputs(cfg, x, seg_ids, W1, b1, W2, b2, l1, l2)
    res = run_bass_kernel_spmd(nc, maps, core_ids=list(range(n_cores)))
    partials = [r["pooled"] for r in res.results]
    out = host_epilogue(cfg, partials, seg_ids, bs, mc)
    return np.ascontiguousarray(out.astype(np.float32))


# revision 11
# speedup vs baseline: 1.5782x; 1.0479x over previous
"""GNN message-passing kernel for Trainium2 (8 NeuronCores, SPMD).

Computation (see np_reference):
  h1 = tanh((A x) @ W1 + b1)      [A = raw adjacency, segsum over dst]
  h2 = tanh((A h1) @ W2 + b2)
  ht = logmap0(proj(h2))          (rowwise scale)
  pooled[seg] = sum over nodes; counts + expmap on host.

Sharding: nodes split contiguously over cores (dst-shard).  The spmm is a
one-hot matmul per 128-edge tile; gathered rows come from gpsimd.dma_gather
with int16 indices (tables chunked to 32768 rows).

v2 layout:
 - one dma_gather per (group, chunk) cell; idx/slot staged per group in one
   DMA each (ACT + SP queues); h1 stored per group.
 - the h1 AllGather is split in two: AG_A (shard rows 0..8191) fires after
   the first 16 L1 groups and overlaps the last 16; AG_B overlaps L2
   phase A (chunks 0,1 read from h1_fullA).  Layer-2 accumulation is split
   per phase with bf16 partials in SBUF.
 - layer 2 accumulates transposed (acc[f, slot]) so its per-block PE
   transpose disappears; pooling is out[hid, seg] with one [128, NSEG128]
   one-hot per block into a single PSUM bank; counts computed on host.
"""

import math
from contextlib import ExitStack

import numpy as np
import ml_dtypes

import concourse.bass as bass
import concourse.tile as tile
import concourse.bacc as bacc
from concourse import mybir

BF16 = mybir.dt.bfloat16
F32 = mybir.dt.float32
I16 = mybir.dt.int16
AF = mybir.ActivationFunctionType
ALU = mybir.AluOpType

MAXNORM = 1.0 - 1e-5
MIN_SS = 1e-15

GRP = 4             # dst blocks (of 128 nodes) per PSUM group
CAP_TILES = 8       # max 128-edge tiles per dma_gather call (HW ucode limit: 1024 idxs)


class Cfg:
    def __init__(self, n_nodes, in_dim, hid, n_seg, n_cores):
        self.N = n_nodes
        self.IN = in_dim
        self.HID = hid
        self.NSEG = n_seg
        self.NC = n_cores
        self.SHARD = n_nodes // n_cores
        assert self.SHARD % 128 == 0
        self.NBLK = self.SHARD // 128
        assert self.NBLK % GRP == 0
        self.NGRP = self.NBLK // GRP
        self.CH = min(32768, n_nodes)
        assert n_nodes % self.CH == 0
        self.NCHUNK = n_nodes // self.CH          # 4
        self.NSEGCH = (n_seg + 127) // 128        # 3
        self.HALF = self.SHARD // 2               # 8192 rows per AG piece


def _edge_fields(cfg, src, dst, permute_src):
    """Per-edge core/block/slot + chunk/idx (optionally with the AG-split
    permutation applied to the src->table-row mapping)."""
    core = dst // cfg.SHARD
    blk = (dst % cfg.SHARD) // 128
    slot = dst % 128
    if permute_src:
        # table row = (srchalf)*N/2 + srccore*HALF + offset
        half = (src % cfg.SHARD) // cfg.HALF
        row = half * (cfg.N // 2) + (src // cfg.SHARD) * cfg.HALF + (src % cfg.HALF)
    else:
        row = src
    chunk = row // cfg.CH
    idx = row % cfg.CH
    return core, blk, slot, chunk, idx


def _layout_layer(cfg, core, blk, slot, chunk, idx, cell_order, force_min):
    """Generic canonical edge-stream builder.

    cell_order: list of (g, k) in processing order (cells iterate b inner).
    force_min: list of lists of k-sets; for each (g, b), each k-set must own
      >= 1 tile (padding goes to the set's first k).
    Returns (ntiles[NGRP, NCHUNK, GRP], per-core idx16 [128, TOT/16] int16,
             per-core slots [128, NTILES] f32, sched).
    """
    NC, NGRP, NCHUNK = cfg.NC, cfg.NGRP, cfg.NCHUNK
    g_all = blk // GRP
    b_all = blk % GRP

    counts = np.zeros((NC, NGRP, NCHUNK, GRP), dtype=np.int64)
    np.add.at(counts, (core, g_all, chunk, b_all), 1)
    mx = counts.max(axis=0)
    ntiles = (mx + 127) // 128
    # force_min: ensure each (g, b) has >=1 tile within each k-set
    for kset in force_min:
        ks = list(kset)
        empty = ntiles[:, ks, :].sum(axis=1) == 0     # [NGRP, GRP]
        sub = ntiles[:, ks[0], :]
        sub[empty] = 1
        ntiles[:, ks[0], :] = sub

    # cell rank in processing order
    cell_rank = np.full((NGRP, NCHUNK, GRP), -1, dtype=np.int64)
    rank = 0
    cells = []      # (g, k, b) in rank order
    for (g, k) in cell_order:
        for b in range(GRP):
            cell_rank[g, k, b] = rank
            cells.append((g, k, b))
            rank += 1
    ncells = rank
    nt_flat = np.zeros(ncells, dtype=np.int64)
    for r, (g, k, b) in enumerate(cells):
        nt_flat[r] = ntiles[g, k, b]
    base = np.zeros(ncells + 1, dtype=np.int64)
    np.cumsum(nt_flat * 128, out=base[1:])
    NTILES = int(nt_flat.sum())
    TOT = NTILES * 128

    edge_rank = cell_rank[g_all, chunk, b_all]
    idx16_cores = []
    slot_cores = []
    for c in range(NC):
        sel = np.nonzero(core == c)[0]
        r = edge_rank[sel]
        order = np.argsort(r, kind="stable")
        sel = sel[order]
        r = r[order]
        # position within cell
        cnt = np.bincount(r, minlength=ncells)
        first = np.zeros(ncells, dtype=np.int64)
        np.cumsum(cnt[:-1], out=first[1:])
        within = np.arange(len(sel)) - first[r]
        pos = base[r] + within
        idx16 = np.zeros(TOT, dtype=np.int16)
        slots = np.full(TOT, -1.0, dtype=np.float32)
        idx16[pos] = idx[sel]
        slots[pos] = slot[sel]
        iw = idx16.reshape(-1, 16).T                  # [16, TOT/16]
        iw = np.tile(iw, (8, 1)).copy()               # [128, TOT/16]
        sl = slots.reshape(NTILES, 128).T.copy()      # [128, NTILES]
        idx16_cores.append(iw.astype(np.int16))
        slot_cores.append(sl.astype(np.float32))

    # schedule: per (g, k) in order -> list of (tcol, b, start, stop)
    # start/stop scope: for each force_min k-set (accumulation scope)
    scope_of_k = {}
    for si, kset in enumerate(force_min):
        for k in kset:
            scope_of_k[k] = si
    tot_b = np.zeros((NGRP, len(force_min), GRP), dtype=np.int64)
    for si, kset in enumerate(force_min):
        for k in kset:
            tot_b[:, si, :] += ntiles[:, k, :]
    seen = np.zeros((NGRP, len(force_min), GRP), dtype=np.int64)
    sched = {}
    tcol = 0
    for (g, k) in cell_order:
        si = scope_of_k[k]
        tiles = []
        for b in range(GRP):
            for _ in range(int(ntiles[g, k, b])):
                st = seen[g, si, b] == 0
                sp = seen[g, si, b] == tot_b[g, si, b] - 1
                tiles.append((tcol, b, bool(st), bool(sp)))
                seen[g, si, b] += 1
                tcol += 1
        sched[(g, k)] = tiles
    return ntiles, idx16_cores, slot_cores, sched, NTILES


def host_prep(cfg, src, dst):
    src = np.asarray(src).astype(np.int64)
    dst = np.asarray(dst).astype(np.int64)
    # layer 1: natural src chunking, k inner per group, one accum scope
    c1 = _edge_fields(cfg, src, dst, permute_src=False)
    order1 = [(g, k) for g in range(cfg.NGRP) for k in range(cfg.NCHUNK)]
    l1 = _layout_layer(cfg, *c1, order1, [range(cfg.NCHUNK)])
    # layer 2: permuted table rows; phase A (k=0,1) then phase B (k=2,3)
    c2 = _edge_fields(cfg, src, dst, permute_src=True)
    order2 = ([(g, k) for g in range(cfg.NGRP) for k in (0, 1)] +
              [(g, k) for g in range(cfg.NGRP) for k in (2, 3)])
    l2 = _layout_layer(cfg, *c2, order2, [(0, 1), (2, 3)])
    return l1, l2


def build(cfg, l1, l2):
    N, IN, HID = cfg.N, cfg.IN, cfg.HID
    NGRP, NCHUNK, NBLK = cfg.NGRP, cfg.NCHUNK, cfg.NBLK
    ntiles1, _, _, sched1, NT1 = l1
    ntiles2, _, _, sched2, NT2 = l2
    NSEGC = cfg.NSEGCH * 128                       # 384
    KIN = IN // 128

    nc = bacc.Bacc("TRN2", target_bir_lowering=False)

    x_d = nc.dram_tensor("x_bf16", [N, IN], BF16, kind="ExternalInput")
    idx1_d = nc.dram_tensor("idx1", [128, NT1 * 8], I16, kind="ExternalInput")
    slot1_d = nc.dram_tensor("slot1", [128, NT1], F32, kind="ExternalInput")
    idx2_d = nc.dram_tensor("idx2", [128, NT2 * 8], I16, kind="ExternalInput")
    slot2_d = nc.dram_tensor("slot2", [128, NT2], F32, kind="ExternalInput")
    segid_d = nc.dram_tensor("segid", [128, NBLK], F32, kind="ExternalInput")
    iota_d = nc.dram_tensor("iota128", [128, 128], BF16, kind="ExternalInput")
    iotas_d = nc.dram_tensor("iota_seg", [128, NSEGC], F32, kind="ExternalInput")
    ident_d = nc.dram_tensor("ident", [128, 128], BF16, kind="ExternalInput")
    w1_d = nc.dram_tensor("W1", [IN, HID], BF16, kind="ExternalInput")
    w2_d = nc.dram_tensor("W2", [HID, HID], BF16, kind="ExternalInput")
    b1_d = nc.dram_tensor("b1row", [1, HID], BF16, kind="ExternalInput")
    b2_d = nc.dram_tensor("b2row", [1, HID], BF16, kind="ExternalInput")

    h1_shard = nc.dram_tensor("h1_shard", [cfg.SHARD, HID], BF16)
    h1_fullA = nc.dram_tensor("h1_fullA", [N // 2, HID], BF16, addr_space="Shared")
    h1_fullB = nc.dram_tensor("h1_fullB", [N // 2, HID], BF16, addr_space="Shared")
    out_d = nc.dram_tensor("pooled", [128, NSEGC], F32, kind="ExternalOutput")

    max_cell1 = int(ntiles1.sum(axis=2).max())
    max_cell2 = int(ntiles2.sum(axis=2).max())
    grp_tiles1 = int(ntiles1.sum(axis=(1, 2)).max())
    grp_tiles2 = int(ntiles2.sum(axis=(1, 2)).max())

    with tile.TileContext(nc) as tc, ExitStack() as ctx:
        const = ctx.enter_context(tc.tile_pool(name="const", bufs=1))
        idxp = ctx.enter_context(tc.tile_pool(name="idxp", bufs=3))
        slotp = ctx.enter_context(tc.tile_pool(name="slotp", bufs=3))
        ebufp = ctx.enter_context(tc.tile_pool(name="ebufp", bufs=4))
        sp = ctx.enter_context(tc.tile_pool(name="sp", bufs=4))
        flshp = ctx.enter_context(tc.tile_pool(name="flshp", bufs=3))
        xtp = ctx.enter_context(tc.tile_pool(name="xtp", bufs=3))
        hp = ctx.enter_context(tc.tile_pool(name="hp", bufs=4))
        h2p = ctx.enter_context(tc.tile_pool(name="h2p", bufs=6))
        partp = ctx.enter_context(tc.tile_pool(name="partp", bufs=1))
        normp = ctx.enter_context(tc.tile_pool(name="normp", bufs=4))
        htp = ctx.enter_context(tc.tile_pool(name="htp", bufs=3))
        h1gp = ctx.enter_context(tc.tile_pool(name="h1gp", bufs=2))

        ps_acc = ctx.enter_context(tc.tile_pool(name="ps_acc", bufs=GRP, space="PSUM"))
        ps_tr = ctx.enter_context(tc.tile_pool(name="ps_tr", bufs=1, space="PSUM"))
        ps_h = ctx.enter_context(tc.tile_pool(name="ps_h", bufs=2, space="PSUM"))
        ps_pool = ctx.enter_context(tc.tile_pool(name="ps_pool", bufs=1, space="PSUM"))

        # ---- constants ----
        iota128 = const.tile([128, 128], BF16)
        nc.sync.dma_start(iota128[:], iota_d[:])
        iotaseg = const.tile([128, NSEGC], F32)
        nc.sync.dma_start(iotaseg[:], iotas_d[:])
        ident = const.tile([128, 128], BF16)
        nc.sync.dma_start(ident[:], ident_d[:])
        segid = const.tile([128, NBLK], F32)
        nc.sync.dma_start(segid[:], segid_d[:])
        w1_sb = [const.tile([128, HID], BF16, tag=f"w1_{k}", name=f"w1_{k}")
                 for k in range(KIN)]
        for k in range(KIN):
            nc.sync.dma_start(w1_sb[k][:], w1_d[k * 128:(k + 1) * 128, :])
        w2_sb = const.tile([128, HID], BF16)
        nc.sync.dma_start(w2_sb[:], w2_d[:])
        b1_sb = const.tile([1, HID], BF16)
        nc.sync.dma_start(b1_sb[:], b1_d[:])
        b2_sb = const.tile([1, HID], BF16)
        nc.sync.dma_start(b2_sb[:], b2_d[:])
        ones1 = const.tile([1, 128], BF16)
        nc.vector.memset(ones1[:], 1.0)

        partA = partp.tile([128, NBLK * HID], BF16)
        h2all = partp.tile([128, NBLK * HID], BF16)
        ss_all = partp.tile([128, NBLK], F32)
        scale_all = partp.tile([128, NBLK], F32)
        pool_ps = ps_pool.tile([128, NSEGC], F32, name="pool_ps")

        def run_group(g, ks, sched, idx_d, slot_d, tables, feat, flip,
                      out_block, grp_cols, cell_cap, preload=None):
            """Process cells (g, k in ks); call out_block(g, b, acc) for
            completed accumulations (only when the scope's stop fired)."""
            tiles_all = [t for k in ks for t in sched[(g, k)]]
            if not tiles_all:
                return
            tcol0 = tiles_all[0][0]
            ntg = len(tiles_all)
            it = idxp.tile([128, grp_cols * 8], I16, tag="it")
            nc.scalar.dma_start(it[:, :ntg * 8],
                                idx_d[:, tcol0 * 8:(tcol0 + ntg) * 8])
            st = slotp.tile([128, grp_cols], F32, tag="st")
            nc.sync.dma_start(st[:, :ntg], slot_d[:, tcol0:tcol0 + ntg])

            accs = {}
            done = []
            for k in ks:
                tiles_k = sched[(g, k)]
                if not tiles_k:
                    continue
                tbl = tables[k]
                for s0 in range(0, len(tiles_k), CAP_TILES):
                    stiles = tiles_k[s0:s0 + CAP_TILES]
                    nidx = len(stiles) * 128
                    c0 = stiles[0][0]
                    eb = ebufp.tile([128, cell_cap * feat], BF16,
                                    tag=f"eb{feat}")
                    nc.gpsimd.dma_gather(
                        out_ap=eb[:, :len(stiles) * feat].rearrange(
                            "p (n f) -> p n f", f=feat),
                        in_ap=tbl,
                        idxs_ap=it[:, (c0 - tcol0) * 8:(c0 - tcol0) * 8 + nidx // 16],
                        num_idxs=nidx,
                        num_idxs_reg=nidx,
                        elem_size=feat,
                    )
                    for j, (tcol, b, st_f, sp_f) in enumerate(stiles):
                        if b not in accs:
                            accs[b] = ps_acc.tile([128, IN], F32, tag="acc",
                                                  name=f"acc{b}")
                            if preload is not None:
                                preload(g, b, accs[b][:, :feat])
                        if preload is not None:
                            st_f = False
                        s_t = sp.tile([128, 128], BF16, tag="s_t")
                        nc.vector.tensor_scalar(
                            s_t[:], iota128[:], st[:, tcol - tcol0:tcol - tcol0 + 1],
                            None, ALU.is_equal)
                        if flip:
                            nc.tensor.matmul(
                                accs[b][:, :feat],
                                eb[:, j * feat:(j + 1) * feat], s_t[:],
                                start=st_f, stop=sp_f)
                        else:
                            nc.tensor.matmul(
                                accs[b][:, :feat],
                                s_t[:], eb[:, j * feat:(j + 1) * feat],
                                start=st_f, stop=sp_f)
                        if sp_f:
                            done.append(b)
            for b in done:
                out_block(g, b, accs[b][:, :feat])

        # ================= layer 1 =================
        x_tables = [x_d[k * cfg.CH:(k + 1) * cfg.CH, :] for k in range(NCHUNK)]

        h1g = [None]

        def l1_block(g, b, agg_ps):
            nb = g * GRP + b
            ax = flshp.tile([128, IN], BF16, tag="ax1")
            nc.scalar.activation(ax[:], agg_ps, AF.Copy)
            h_ps = ps_h.tile([128, HID], F32, tag="hps", name="h_ps")
            for h in range(KIN):
                t_ps = ps_tr.tile([128, 128], BF16, tag="tps")
                nc.tensor.transpose(t_ps[:], ax[:, h * 128:(h + 1) * 128], ident[:])
                xt = xtp.tile([128, 128], BF16, tag="xt")
                nc.scalar.activation(xt[:], t_ps[:], AF.Copy)
                nc.tensor.matmul(h_ps[:], xt[:], w1_sb[h][:],
                                 start=(h == 0), stop=False)
            nc.tensor.matmul(h_ps[:], ones1[:], b1_sb[:], start=False, stop=True)
            nc.scalar.activation(h1g[0][:, b * HID:(b + 1) * HID], h_ps[:], AF.Tanh)

        for g in range(NGRP):
            h1g[0] = h1gp.tile([128, GRP * HID], BF16, tag="h1g", name="h1g")
            run_group(g, range(NCHUNK), sched1, idx1_d, slot1_d, x_tables,
                      IN, False, l1_block, grp_tiles1,
                      min(max_cell1, CAP_TILES))
            nc.sync.dma_start(
                h1_shard[g * GRP * 128:(g + 1) * GRP * 128, :].rearrange(
                    "(b s) h -> s b h", s=128),
                h1g[0][:].rearrange("s (b h) -> s b h", h=HID))
            if g == NGRP // 2 - 1:
                nc.gpsimd.collective_compute(
                    "AllGather", ALU.bypass,
                    ins=[h1_shard[0:cfg.HALF, :].opt()],
                    outs=[h1_fullA.ap().opt()],
                    replica_groups=[list(range(cfg.NC))])
        nc.gpsimd.collective_compute(
            "AllGather", ALU.bypass,
            ins=[h1_shard[cfg.HALF:cfg.SHARD, :].opt()],
            outs=[h1_fullB.ap().opt()],
            replica_groups=[list(range(cfg.NC))])

        # ================= layer 2 =================
        h_tables = {0: h1_fullA[0:cfg.CH, :], 1: h1_fullA[cfg.CH:2 * cfg.CH, :],
                    2: h1_fullB[0:cfg.CH, :], 3: h1_fullB[cfg.CH:2 * cfg.CH, :]}

        def l2a_block(g, b, agg_ps):
            nb = g * GRP + b
            nc.scalar.activation(partA[:, nb * HID:(nb + 1) * HID], agg_ps,
                                 AF.Copy)

        for g in range(NGRP):
            run_group(g, (0, 1), sched2, idx2_d, slot2_d, h_tables,
                      HID, True, l2a_block, grp_tiles2,
                      min(max_cell2, CAP_TILES))

        # phase B: spmm (PSUM preloaded from partA) + tanh + norm-sq only
        def l2b_preload(g, b, acc):
            nb = g * GRP + b
            nc.tensor.matmul(acc, ident[:], partA[:, nb * HID:(nb + 1) * HID],
                             start=True, stop=False)

        def l2b_block(g, b, agg_ps):
            nb = g * GRP + b
            a2t = flshp.tile([128, HID], BF16, tag="a2t")
            nc.scalar.activation(a2t[:], agg_ps, AF.Copy)
            h_ps = ps_h.tile([128, HID], F32, tag="hps", name="h_ps")
            nc.tensor.matmul(h_ps[:], a2t[:], w2_sb[:], start=True, stop=False)
            nc.tensor.matmul(h_ps[:], ones1[:], b2_sb[:], start=False, stop=True)
            h2b = h2all[:, nb * HID:(nb + 1) * HID]
            nc.scalar.activation(h2b, h_ps[:], AF.Tanh)
            sq = hp.tile([128, HID], F32, tag="sq")
            nc.vector.tensor_mul(sq[:], h2b, h2b)
            nc.vector.tensor_reduce(ss_all[:, nb:nb + 1], sq[:],
                                    mybir.AxisListType.X, ALU.add)

        for g in range(NGRP):
            run_group(g, (2, 3), sched2, idx2_d, slot2_d, h_tables,
                      HID, True, l2b_block, grp_tiles2,
                      min(max_cell2, CAP_TILES), preload=l2b_preload)

        # vectorized scale = artanh(min(sqrt(max(ss,MIN)), MAXNORM))/sqrt(..)
        na = normp.tile([128, NBLK], F32, tag="na")
        nb_t = normp.tile([128, NBLK], F32, tag="nb")
        nc.vector.tensor_scalar_max(na[:], ss_all[:], MIN_SS)
        nc.scalar.activation(nb_t[:], na[:], AF.Sqrt)
        nc.vector.tensor_scalar_min(na[:], nb_t[:], MAXNORM)
        one_m = normp.tile([128, NBLK], F32, tag="om")
        nc.vector.tensor_scalar(one_m[:], na[:], -1.0, 1.0, ALU.mult, ALU.add)
        one_p = normp.tile([128, NBLK], F32, tag="op")
        nc.vector.tensor_scalar_add(one_p[:], na[:], 1.0)
        rcp = normp.tile([128, NBLK], F32, tag="rc")
        nc.vector.reciprocal(rcp[:], one_m[:])
        rat = normp.tile([128, NBLK], F32, tag="ra")
        nc.vector.tensor_mul(rat[:], one_p[:], rcp[:])
        lg = normp.tile([128, NBLK], F32, tag="lg")
        nc.scalar.activation(lg[:], rat[:], AF.Ln)
        rcpn = normp.tile([128, NBLK], F32, tag="rn")
        nc.vector.reciprocal(rcpn[:], nb_t[:])
        nc.vector.tensor_mul(rcpn[:], lg[:], rcpn[:])
        nc.vector.tensor_scalar_mul(scale_all[:], rcpn[:], 0.5)

        # pooling sweep: scaled one-hot (is_equal then *scale) as moving rhs
        for nb in range(NBLK):
            sg = sp.tile([128, NSEGC], BF16, tag="sg")
            nc.vector.tensor_scalar(sg[:], iotaseg[:], segid[:, nb:nb + 1],
                                    scale_all[:, nb:nb + 1],
                                    ALU.is_equal, ALU.mult)
            nc.tensor.matmul(pool_ps[:], h2all[:, nb * HID:(nb + 1) * HID],
                             sg[:], start=(nb == 0), stop=(nb == NBLK - 1))

        po = htp.tile([128, NSEGC], F32, tag="po")
        nc.vector.tensor_copy(po[:], pool_ps[:])
        nc.sync.dma_start(out_d[:], po[:])

    nc.compile()
    return nc


def host_inputs(cfg, x, seg_ids, W1, b1, W2, b2, l1, l2):
    N, IN, HID = cfg.N, cfg.IN, cfg.HID
    _, idx1_cores, slot1_cores, _, _ = l1
    _, idx2_cores, slot2_cores, _, _ = l2
    x_bf16 = np.ascontiguousarray(x.astype(ml_dtypes.bfloat16))
    iota128 = np.tile(np.arange(128, dtype=np.float32), (128, 1)).astype(ml_dtypes.bfloat16)
    iotaseg = np.tile(np.arange(cfg.NSEGCH * 128, dtype=np.float32), (128, 1)).astype(np.float32)
    ident = np.eye(128, dtype=np.float32).astype(ml_dtypes.bfloat16)
    w1 = np.ascontiguousarray(W1.astype(ml_dtypes.bfloat16))
    w2 = np.ascontiguousarray(W2.astype(ml_dtypes.bfloat16))
    b1r = np.asarray(b1, np.float32).reshape(1, -1).astype(ml_dtypes.bfloat16)
    b2r = np.asarray(b2, np.float32).reshape(1, -1).astype(ml_dtypes.bfloat16)
    seg = np.asarray(seg_ids, np.float32)
    maps = []
    for c in range(cfg.NC):
        segc = seg[c * cfg.SHARD:(c + 1) * cfg.SHARD].reshape(cfg.NBLK, 128).T
        maps.append({
            "x_bf16": x_bf16,
            "idx1": idx1_cores[c], "slot1": slot1_cores[c],
            "idx2": idx2_cores[c], "slot2": slot2_cores[c],
            "segid": np.ascontiguousarray(segc),
            "iota128": iota128,
            "iota_seg": np.ascontiguousarray(iotaseg),
            "ident": ident,
            "W1": w1, "W2": w2, "b1row": b1r, "b2row": b2r,
        })
    return maps


def host_epilogue(cfg, partials, seg_ids, batch_size, max_comments):
    """partials: list of per-core [128, NSEGC] f32 (hid x seg)."""
    acc = np.zeros_like(partials[0], dtype=np.float64)
    for p in partials:
        acc += p.astype(np.float64)
    sums = acc.T[:cfg.NSEG, :].astype(np.float32)        # [NSEG, HID]
    counts = np.bincount(np.asarray(seg_ids).astype(np.int64),
                         minlength=cfg.NSEG).astype(np.float32)
    agg = sums / np.maximum(counts, 1.0)[:, None]
    ss = np.maximum(np.sum(agg * agg, axis=1), MIN_SS).astype(np.float32)
    norm = np.sqrt(ss)
    y = agg * (np.tanh(norm) / norm)[:, None]
    ssy = np.maximum(np.sum(y * y, axis=1), MIN_SS).astype(np.float32)
    ny = np.sqrt(ssy)
    f = np.where(ny > MAXNORM, MAXNORM / ny, 1.0).astype(np.float32)
    y = y * f[:, None]
    return y.reshape(int(batch_size), int(max_comments), cfg.HID)


# ---------------- numpy reference (for arbitrary sizes) ----------------

def np_reference(x, src, dst, seg_ids, W1, b1, W2, b2, batch_size, max_comments):
    n = x.shape[0]

    def seg_sum(vals, ids, nseg):
        out = np.zeros((nseg, vals.shape[1]), np.float32)
        np.add.at(out, ids, vals)
        return out

    def rownorm(v):
        return np.sqrt(np.maximum(np.sum(v * v, axis=1, keepdims=True), MIN_SS))

    def proj(v):
        nn = rownorm(v)
        return np.where(nn > MAXNORM, v / nn * MAXNORM, v)

    def logmap0(v):
        nn = rownorm(v)
        arg = np.minimum(nn, 1 - 1e-7)
        return v * np.arctanh(arg) / nn

    def expmap0(v):
        nn = rownorm(v)
        return v * np.tanh(nn) / nn

    h = np.tanh(seg_sum(x[src] @ W1, dst, n) + b1)
    h = np.tanh(seg_sum(h[src] @ W2, dst, n) + b2)
    h = logmap0(proj(h))
    nseg = int(batch_size) * int(max_comments)
    sums = seg_sum(h, seg_ids, nseg)
    counts = np.zeros(nseg, np.float32)
    np.add.at(counts, seg_ids, 1.0)
    agg = sums / np.maximum(counts, 1.0)[:, None]
    agg = proj(expmap0(agg))
    return agg.reshape(int(batch_size), int(max_comments), -1)


# ====================================================================
# Harness entry point: kernel(**inputs) -> np.ndarray
# ====================================================================

_CACHE = {}


def kernel(x, src, dst, seg_ids, W1, b1, W2, b2, batch_size, max_comments):
    """Full-input GNN ComEnc kernel on 8 Trainium2 NeuronCores."""
    from concourse.bass_utils import run_bass_kernel_spmd

    x = np.asarray(x, dtype=np.float32)
    src = np.asarray(src).astype(np.int64)
    dst = np.asarray(dst).astype(np.int64)
    seg_ids = np.asarray(seg_ids).astype(np.int64)
    W1 = np.asarray(W1, dtype=np.float32)
    b1 = np.asarray(b1, dtype=np.float32)
    W2 = np.asarray(W2, dtype=np.float32)
    b2 = np.asarray(b2, dtype=np.float32)
    bs = int(np.asarray(batch_size))
    mc = int(np.asarray(max_comments))

    n_nodes, in_dim = x.shape
    hid = W1.shape[1]
    nseg = bs * mc
    n_cores = 8

    cfg = Cfg(n_nodes, in_dim, hid, nseg, n_cores)
    l1, l2 = host_prep(cfg, src, dst)

    key = (n_nodes, in_dim, hid, nseg, l1[0].tobytes(), l2[0].tobytes())
    if key in _CACHE:
        nc = _CACHE[key]
    else:
        nc = build(cfg, l1, l2)
        _CACHE.clear()
        _CACHE[key] = nc

    maps = host_inputs(cfg, x, seg_ids, W1, b1, W2, b2, l1, l2)
    res = run_bass_kernel_spmd(nc, maps, core_ids=list(range(n_cores)))
    partials = [r["pooled"] for r in res.results]
    out = host_epilogue(cfg, partials, seg_ids, bs, mc)
    return np.ascontiguousarray(out.astype(np.float32))
